# revision 34
# baseline (speedup 1.0000x reference)
"""Multi-head attention (16 heads, d=64, d_model=1024, SL=2048, BS=2) on 8
Trainium2 NeuronCores.

Sharding: core c handles batch b = c // 4 and heads [4*(c%4), 4*(c%4)+4).
Each core computes a partial output y_c[2048, 1024] (its 4 heads' contribution
through Wo for its batch); the host sums the 4 partials per batch.

Host-side prep feeds activations TRANSPOSED ([d_model, seq]) so every on-chip
matmul has its contraction dim on partitions; no on-chip transposes needed.

Per-core dataflow (all matmul moving streams >= 256 cols so weight loads hide):
  V[2048,256] bf16 with interleaved ones cols ([V_h | 1] per head, 65 cols)
  Q^T,K^T [256,2048] f32r (psum-accumulated over 8 d_model chunks)
  per (q-chunk 512, head): S^T[k,q] psum tiles -> exp (ACT, no max-subtraction:
  |scores| < ~25 so fp32 exp is exact-enough) -> P^T bf16 -> attnU^T[65,512]
  accumulated over 16 k-tiles; row 64 = softmax denominator l.
  evac au->SBUF (frees psum fast), recip(l) -> Pool broadcast -> DVE mul -> A^T
  O-proj: y[q,1024] += A^T-chunk.T @ Wo^T-chunk (wo bf16).

Scheduling (v9):
  - x inputs as contiguous [128,2048] chunk DMAs on the HWDGE (sync) queue
    (4KB rows: HW DMA cost is per-descriptor=per-row, so contiguous large
    rows are far cheaper than strided small-run DMAs); weights pre-laid-out
    on the host to SBUF shape so each loads as ONE contiguous DMA (scalar
    queue). Y out on HWDGE, bf16 (host accumulates partials in f32).
  - emission: K hp0 b0 + Q hp0 b0 lead-in, then 8 attention units
    pair-major. Unit 0's g-loop weaves in the V projection (all 16 k-tiles)
    and the rest of K hp0; K hp1 groups ride units 2-3; each Q proj group is
    emitted at g7 of the unit before its first use (ahead of that unit's
    normalize chain so its PSUM-evac copy isn't queued behind it); oproj(qc)
    tiles are woven into unit (5+qc)'s g-loop, one unit after the AT writes
    they read; oproj(qc3) trails.
  - engines: ALL PSUM evacuations on DVE (Pool cannot touch PSUM on HW);
    Pool only runs partition_broadcasts; normalize emitted in phases
    (l-copies, recips, pbs, muls) so each engine queue blocks only at its
    tail, never mid-chain.
  - PSUM banks: s double-buffered (2x2), au pool 2, proj-acc/oproj pool 2.
  - Exp activation table preloaded at t=0 via a dummy exp.
"""

import os
import sys
for _p in ("/opt/trn_rl_repo", "/root/.axon_site/_ro/trn_rl_repo"):
    if os.path.isdir(_p) and _p not in sys.path:
        sys.path.insert(0, _p)

import numpy as np

import concourse.bass as bass
import concourse.tile as tile
from concourse import bacc, mybir
from concourse.bass_utils import run_bass_kernel_spmd

N_CORES = 8
SL = 2048
BS = 2
DM = 1024          # d_model
H = 16             # total heads
DH = 64            # head dim
HPC = 4            # heads per core
IC = HPC * DH      # per-core inner dim = 256
F32 = mybir.dt.float32
BF16 = mybir.dt.bfloat16
F32R = mybir.dt.float32r
Exp = mybir.ActivationFunctionType.Exp

N_DMC = DM // 128          # 8 d_model chunks
N_KT = SL // 128           # 16 k tiles
N_QC = SL // 512           # 4 q chunks (= x DMA blocks)
VW = 65                    # V columns per head incl. ones column
VBLK = HPC * VW            # 260 V columns per k-tile block


def build_kernel(reps=1):
    nc = bacc.Bacc("TRN2", target_bir_lowering=False, debug=False,
                   num_devices=N_CORES)
    qT = nc.dram_tensor("qT", [DM, SL], BF16, kind="ExternalInput").ap()
    kT = nc.dram_tensor("kT", [DM, SL], BF16, kind="ExternalInput").ap()
    vT = nc.dram_tensor("vT", [DM, SL], BF16, kind="ExternalInput").ap()
    # wq/wk/wv are pre-laid-out on the host in SBUF shape [128, 8*256]
    # (partition-major) so each loads with ONE contiguous 4KB-row DMA
    wqT = nc.dram_tensor("wqT", [128, N_DMC * IC], BF16,
                         kind="ExternalInput").ap()
    wkT = nc.dram_tensor("wkT", [128, N_DMC * IC], BF16,
                         kind="ExternalInput").ap()
    wvT = nc.dram_tensor("wvT", [128, N_DMC * IC], BF16,
                         kind="ExternalInput").ap()
    woT = nc.dram_tensor("woT", [IC, DM], BF16, kind="ExternalInput").ap()
    Y = nc.dram_tensor("Y", [SL, DM], BF16, kind="ExternalOutput").ap()

    with tile.TileContext(nc) as tc:
        _build_body(nc, tc, qT, kT, vT, wqT, wkT, wvT, woT, Y, reps)
    nc.compile()
    return nc


def _build_body(nc, tc, qT, kT, vT, wqT, wkT, wvT, woT, Y, reps=1):
    import contextlib
    ctx = contextlib.ExitStack()
    with ctx:
        wpool = ctx.enter_context(tc.tile_pool(name="w", bufs=1))
        xin = ctx.enter_context(tc.tile_pool(name="xin", bufs=24))
        qk = ctx.enter_context(tc.tile_pool(name="qk", bufs=1))
        vpool = ctx.enter_context(tc.tile_pool(name="v", bufs=1))
        ptp = ctx.enter_context(tc.tile_pool(name="pt", bufs=4))
        atp = ctx.enter_context(tc.tile_pool(name="at", bufs=1))
        ypool = ctx.enter_context(tc.tile_pool(name="y", bufs=2))
        misc = ctx.enter_context(tc.tile_pool(name="misc", bufs=3))
        ps = ctx.enter_context(tc.tile_pool(name="ps", bufs=2, space="PSUM"))
        psu = ctx.enter_context(tc.tile_pool(name="psu", bufs=2, space="PSUM"))
        psy = ctx.enter_context(tc.tile_pool(name="psy", bufs=2, space="PSUM"))

        w_sb = {}
        for name in ("wq", "wk", "wv"):
            w_sb[name] = wpool.tile([128, N_DMC * IC], BF16, tag=name,
                                    name=name)
        wo_sb = [wpool.tile([128, DM], BF16, tag=f"wo{i}", name=f"wo{i}")
                 for i in range(2)]

        ones_f32 = misc.tile([128, DH], F32, tag="ones_f32")
        nc.vector.memset(ones_f32[:], 1.0)
        # preload the Exp table while DMAs run
        warm = misc.tile([1, 2], F32, tag="warm")
        nc.vector.memset(warm[:], 0.0)
        nc.scalar.activation(warm[:], warm[:], Exp)

        # ---- long-lived activations ----
        QT = [qk.tile([128, SL], F32R, tag=f"qt{p}", name=f"qt{p}")
              for p in range(2)]
        KT = [qk.tile([128, SL], F32R, tag=f"kt{p}", name=f"kt{p}")
              for p in range(2)]
        AT = [atp.tile([128, SL], BF16, tag=f"at{p}", name=f"at{p}")
              for p in range(2)]
        V = vpool.tile([128, N_KT * VBLK], BF16, tag="vsb")
        for h in range(HPC):
            nc.vector.tensor_copy(V[:, h * VW + 64::VBLK],
                                  ones_f32[:, 0:N_KT])

        for _rep in range(reps):
            # ---- x inputs: contiguous [128,2048] chunk DMAs (4KB rows keep
            # the HW descriptor count low; strided 1KB-run DMAs measured far
            # slower on HW than the sim models). Landing order: kT, qT, vT.
            def x_chunks(xdram, pref):
                ts = []
                for c in range(N_DMC):
                    t = xin.tile([128, SL], BF16, tag="xin",
                                 name=f"{pref}{c}")
                    nc.sync.dma_start(out=t[:],
                                      in_=xdram[c * 128:(c + 1) * 128, :])
                    ts.append(t)
                return ts

            w_dram = {"wq": wqT, "wk": wkT, "wv": wvT}

            def w_load(name):
                if name == "wo":
                    for i in range(2):
                        nc.scalar.dma_start(out=wo_sb[i][:],
                                            in_=woT[i * 128:(i + 1) * 128, :])
                    return
                nc.scalar.dma_start(out=w_sb[name][:], in_=w_dram[name][:])

            w_load("wk")
            w_load("wq")
            w_load("wv")
            w_load("wo")
            kx = x_chunks(kT, "kx")
            qx = x_chunks(qT, "qx")
            vx = x_chunks(vT, "vx")

            # ---- projection group emitters ----
            def qk_group(xt, wname, out_tiles, hp, b):
                acc = psy.tile([128, 512], F32, tag="yp", name=f"acc_{wname}{hp}_{b}")
                for c in range(N_DMC):
                    nc.tensor.matmul(
                        acc[:],
                        w_sb[wname][:, c * IC + hp * 128:
                                    c * IC + (hp + 1) * 128],
                        xt[c][:, b * 512:(b + 1) * 512],
                        start=(c == 0), stop=(c == N_DMC - 1))
                # always DVE: Pool carries partition_broadcasts whose waits
                # would delay a queued proj copy past the next unit's scores
                nc.vector.tensor_copy(out_tiles[hp][:, b * 512:(b + 1) * 512],
                                      acc[:])

            def v_group(kt):
                acc = psy.tile([128, 512], F32, tag="yp", name=f"acc_v{kt}")
                for c in range(N_DMC):
                    nc.tensor.matmul(
                        acc[:, 0:IC],
                        vx[c][:, kt * 128:(kt + 1) * 128],
                        w_sb["wv"][:, c * IC:(c + 1) * IC],
                        start=(c == 0), stop=(c == N_DMC - 1))
                # one strided copy: 4 heads' 64-col blocks into 65-col slots
                nc.vector.tensor_copy(
                    V[:, kt * VBLK:(kt + 1) * VBLK]
                    .rearrange("p (h w) -> p h w", h=HPC)[:, :, 0:64],
                    acc[:, 0:IC].rearrange("p (h w) -> p h w", h=HPC))

            # ---- lead-in projections (rest of K hp0 woven into unit 0) ----
            qk_group(kx, "wk", KT, 0, 0)
            qk_group(qx, "wq", QT, 0, 0)

            def attn_unit(qc, pair, g_insert=None):
                au = [psu.tile([VW, 512], F32, tag="accu", name=f"au{hl}")
                      for hl in range(2)]
                for g in range(N_KT // 2):
                    pts = []
                    for hl in range(2):
                        s = ps.tile([128, 1024], F32, tag="sgrp")
                        for j in range(2):
                            kt = 2 * g + j
                            nc.tensor.matmul(
                                s[:, j * 512:(j + 1) * 512],
                                KT[pair][hl * 64:(hl + 1) * 64,
                                         kt * 128:(kt + 1) * 128],
                                QT[pair][hl * 64:(hl + 1) * 64,
                                         qc * 512:(qc + 1) * 512],
                                start=True, stop=True)
                        p = ptp.tile([128, 1024], BF16, tag="pt")
                        nc.scalar.activation(p[:], s[:], Exp)
                        pts.append(p)
                    if g_insert is not None:
                        g_insert(g)
                    for hl in range(2):
                        h = pair * 2 + hl
                        for j in range(2):
                            kt = 2 * g + j
                            nc.tensor.matmul(
                                au[hl][:],
                                V[:, kt * VBLK + h * VW:
                                  kt * VBLK + (h + 1) * VW],
                                pts[hl][:, j * 512:(j + 1) * 512],
                                start=(kt == 0), stop=(kt == N_KT - 1))
                # normalize from PSUM (Pool can't read PSUM on HW, so all
                # PSUM reads stay on DVE). Phased emission (l-copies, recips,
                # pbs, muls) so each engine queue blocks only at its tail.
                # NB: reciprocal_approx_fast reading PSUM directly returns
                # garbage on HW — the l row must be staged through SBUF.
                lsbs, rcs, rbs = [], [], []
                for hl in range(2):
                    l_sb = misc.tile([1, 512], F32, tag="lsb")
                    nc.vector.tensor_copy(l_sb[:], au[hl][64:65, :])
                    lsbs.append(l_sb)
                for hl in range(2):
                    rc = misc.tile([1, 512], F32, tag="rc")
                    nc.vector.reciprocal_approx_fast(out=rc[:],
                                                     in_=lsbs[hl][:])
                    rcs.append(rc)
                for hl in range(2):
                    rb = misc.tile([64, 512], F32, tag="rb")
                    nc.gpsimd.partition_broadcast(rb[:], rcs[hl][:])
                    rbs.append(rb)
                for hl in range(2):
                    nc.vector.tensor_mul(
                        AT[pair][hl * 64:(hl + 1) * 64,
                                 qc * 512:(qc + 1) * 512],
                        au[hl][0:64, :], rbs[hl][:])

            def oproj_qt(qt):
                y_sb = ypool.tile([128, DM], BF16, tag="ysb")
                for mh in range(2):
                    yp = psy.tile([128, 512], F32, tag="yp")
                    for ich in range(2):
                        nc.tensor.matmul(
                            yp[:],
                            AT[ich][:, qt * 128:(qt + 1) * 128],
                            wo_sb[ich][:, mh * 512:(mh + 1) * 512],
                            start=(ich == 0), stop=(ich == 1))
                    nc.vector.tensor_copy(y_sb[:, mh * 512:(mh + 1) * 512],
                                          yp[:])
                nc.sync.dma_start(out=Y[qt * 128:(qt + 1) * 128, :],
                                  in_=y_sb[:])

            # g-indexed PE inserts per unit:
            # unit 0 weaves V proj (paced by vT block DMAs); units 2,3 carry
            # K hp1 groups; units 5..7 carry oproj of qc-1 (one unit of slack
            # after the AT writes they read).
            def u0_ins(g):
                # K hp0 block g+1 must land before scores g=2(g+1) reads it
                if g < 3:
                    qk_group(kx, "wk", KT, 0, g + 1)
                v_group(2 * g)
                v_group(2 * g + 1)

            # Q/K proj groups are emitted at g7 of the PRECEDING unit so
            # their DVE psum-evac copies queue ahead of that unit's
            # normalize chain (otherwise the psy rotation stalls the next
            # unit's oproj/scores behind the chain).
            g_ins = {
                0: {g: [lambda g=g: u0_ins(g)] for g in range(8)},
                2: {4: [lambda: qk_group(kx, "wk", KT, 1, 0)],
                    6: [lambda: qk_group(kx, "wk", KT, 1, 1)]},
                3: {2: [lambda: qk_group(kx, "wk", KT, 1, 2)],
                    5: [lambda: qk_group(kx, "wk", KT, 1, 3)]},
                5: {g: [lambda g=g: oproj_qt(0 * 4 + g // 2)]
                    for g in (1, 3, 5, 7)},
                6: {g: [lambda g=g: oproj_qt(1 * 4 + g // 2)]
                    for g in (1, 3, 5, 7)},
                7: {g: [lambda g=g: oproj_qt(2 * 4 + g // 2)]
                    for g in (1, 3, 5, 7)},
            }
            q_weave = {
                0: lambda: qk_group(qx, "wq", QT, 0, 1),
                1: lambda: qk_group(qx, "wq", QT, 0, 2),
                2: lambda: qk_group(qx, "wq", QT, 0, 3),
                3: lambda: qk_group(qx, "wq", QT, 1, 0),
                4: lambda: qk_group(qx, "wq", QT, 1, 1),
                5: lambda: qk_group(qx, "wq", QT, 1, 2),
                6: lambda: qk_group(qx, "wq", QT, 1, 3),
            }
            for i, fn in q_weave.items():
                g_ins.setdefault(i, {}).setdefault(7, []).append(fn)

            for i in range(8):
                pair, qc = i // 4, i % 4
                gmap = g_ins.get(i, {})
                attn_unit(qc, pair,
                          g_insert=(lambda g, gm=gmap:
                                    [fn() for fn in gm.get(g, ())]))
            for qt in range(12, 16):
                oproj_qt(qt)


_NC_CACHE = None


def _get_nc():
    global _NC_CACHE
    if _NC_CACHE is None:
        _NC_CACHE = build_kernel()
    return _NC_CACHE


def make_in_maps(query, keys, values, Wq, Wk, Wv, Wo):
    query = np.ascontiguousarray(query, dtype=np.float32)
    keys = np.ascontiguousarray(keys, dtype=np.float32)
    values = np.ascontiguousarray(values, dtype=np.float32)
    import ml_dtypes
    bf16 = ml_dtypes.bfloat16
    xTs = {}
    for b in range(BS):
        xTs[b] = (
            np.ascontiguousarray(query[:, b, :].T.astype(bf16)),
            np.ascontiguousarray(keys[:, b, :].T.astype(bf16)),
            np.ascontiguousarray(values[:, b, :].T.astype(bf16)),
        )
    def w_sbuf_layout(wT):
        # [1024 dm, 256 ic] -> SBUF image [128, 8*256]: chunk c at cols c*256
        return np.ascontiguousarray(
            wT.reshape(N_DMC, 128, IC).transpose(1, 0, 2).reshape(128, -1))

    wTs = {}
    for g in range(N_CORES // BS):
        sl = slice(g * IC, (g + 1) * IC)
        wTs[g] = (
            w_sbuf_layout(np.asarray(Wq, np.float32)[sl, :].T.astype(bf16)),
            w_sbuf_layout(np.asarray(Wk, np.float32)[sl, :].T.astype(bf16)),
            w_sbuf_layout(np.asarray(Wv, np.float32)[sl, :].T.astype(bf16)),
            np.ascontiguousarray(np.asarray(Wo, np.float32)[:, sl].T.astype(bf16)),
        )
    in_maps = []
    for c in range(N_CORES):
        b, g = c // 4, c % 4
        qTb, kTb, vTb = xTs[b]
        wq, wk, wv, wo = wTs[g]
        in_maps.append({"qT": qTb, "kT": kTb, "vT": vTb,
                        "wqT": wq, "wkT": wk, "wvT": wv, "woT": wo})
    return in_maps


def assemble_output(results):
    out = np.zeros((SL, BS, DM), dtype=np.float32)
    for c in range(N_CORES):
        b = c // 4
        out[:, b, :] += np.asarray(results[c]["Y"], dtype=np.float32)
    return out


def kernel(query, keys, values, Wq, Wk, Wv, Wo):
    nc = _get_nc()
    in_maps = make_in_maps(query, keys, values, Wq, Wk, Wv, Wo)
    res = run_bass_kernel_spmd(nc, in_maps, list(range(N_CORES)))
    return assemble_output(res.results)


# revision 42
# speedup vs baseline: 1.2052x; 1.2052x over previous
"""Multi-head attention (16 heads, d=64, d_model=1024, SL=2048, BS=2) on 8
Trainium2 NeuronCores.

Sharding: core c handles batch b = c // 4 and heads [4*(c%4), 4*(c%4)+4).
Each core computes a partial output y_c[2048, 1024] (its 4 heads' contribution
through Wo for its batch); the host sums the 4 partials per batch.

Host-side prep feeds activations TRANSPOSED ([d_model, seq]) so every on-chip
matmul has its contraction dim on partitions; no on-chip transposes needed.

Per-core dataflow (all matmul moving streams >= 256 cols so weight loads hide):
  V[2048,256] bf16 with interleaved ones cols ([V_h | 1] per head, 65 cols)
  Q^T,K^T [256,2048] f32r (psum-accumulated over 8 d_model chunks)
  per (q-chunk 512, head): S^T[k,q] psum tiles -> exp (ACT, no max-subtraction:
  |scores| < ~25 so fp32 exp is exact-enough) -> P^T bf16 -> attnU^T[65,512]
  accumulated over 16 k-tiles; row 64 = softmax denominator l.
  evac au->SBUF (frees psum fast), recip(l) -> Pool broadcast -> DVE mul -> A^T
  O-proj: y[q,1024] += A^T-chunk.T @ Wo^T-chunk (wo bf16).

Scheduling (v9):
  - x inputs as contiguous [128,2048] chunk DMAs on the HWDGE (sync) queue
    (4KB rows: HW DMA cost is per-descriptor=per-row, so contiguous large
    rows are far cheaper than strided small-run DMAs); weights pre-laid-out
    on the host to SBUF shape so each loads as ONE contiguous DMA (scalar
    queue). Y out on HWDGE, bf16 (host accumulates partials in f32).
  - emission: K hp0 b0 + Q hp0 b0 lead-in, then 8 attention units
    pair-major. Unit 0's g-loop weaves in the V projection (all 16 k-tiles)
    and the rest of K hp0; K hp1 groups ride units 2-3; each Q proj group is
    emitted at g7 of the unit before its first use (ahead of that unit's
    normalize chain so its PSUM-evac copy isn't queued behind it); oproj(qc)
    tiles are woven into unit (5+qc)'s g-loop, one unit after the AT writes
    they read; oproj(qc3) trails.
  - engines: ALL PSUM evacuations on DVE (Pool cannot touch PSUM on HW);
    Pool only runs partition_broadcasts; normalize emitted in phases
    (l-copies, recips, pbs, muls) so each engine queue blocks only at its
    tail, never mid-chain.
  - PSUM banks: s double-buffered (2x2), au pool 2, proj-acc/oproj pool 2.
  - Exp activation table preloaded at t=0 via a dummy exp.
"""

import os
import sys
for _p in ("/opt/trn_rl_repo", "/root/.axon_site/_ro/trn_rl_repo"):
    if os.path.isdir(_p) and _p not in sys.path:
        sys.path.insert(0, _p)

import numpy as np

import concourse.bass as bass
import concourse.tile as tile
from concourse import bacc, mybir
from concourse.bass_utils import run_bass_kernel_spmd

N_CORES = 8
SL = 2048
BS = 2
DM = 1024          # d_model
H = 16             # total heads
DH = 64            # head dim
HPC = 4            # heads per core
IC = HPC * DH      # per-core inner dim = 256
F32 = mybir.dt.float32
BF16 = mybir.dt.bfloat16
F32R = mybir.dt.float32r
Exp = mybir.ActivationFunctionType.Exp

N_DMC = DM // 128          # 8 d_model chunks
N_KT = SL // 128           # 16 k tiles
N_QC = SL // 512           # 4 q chunks (= x DMA blocks)
VW = 65                    # V columns per head incl. ones column
VBLK = HPC * VW            # 260 V columns per k-tile block


def build_kernel(reps=1):
    nc = bacc.Bacc("TRN2", target_bir_lowering=False, debug=False,
                   num_devices=N_CORES)
    qT = nc.dram_tensor("qT", [DM, SL], BF16, kind="ExternalInput").ap()
    kT = nc.dram_tensor("kT", [DM, SL], BF16, kind="ExternalInput").ap()
    vT = nc.dram_tensor("vT", [DM, SL], BF16, kind="ExternalInput").ap()
    # wq/wk/wv are pre-laid-out on the host in SBUF shape [128, 8*256]
    # (partition-major) so each loads with ONE contiguous 4KB-row DMA
    wqT = nc.dram_tensor("wqT", [128, N_DMC * IC], BF16,
                         kind="ExternalInput").ap()
    wkT = nc.dram_tensor("wkT", [128, N_DMC * IC], BF16,
                         kind="ExternalInput").ap()
    wvT = nc.dram_tensor("wvT", [128, N_DMC * IC], BF16,
                         kind="ExternalInput").ap()
    woT = nc.dram_tensor("woT", [IC, DM], BF16, kind="ExternalInput").ap()
    Y = nc.dram_tensor("Y", [SL, DM], BF16, kind="ExternalOutput").ap()

    with tile.TileContext(nc) as tc:
        _build_body(nc, tc, qT, kT, vT, wqT, wkT, wvT, woT, Y, reps)
    nc.compile()
    return nc


def _build_body(nc, tc, qT, kT, vT, wqT, wkT, wvT, woT, Y, reps=1):
    import contextlib
    ctx = contextlib.ExitStack()
    with ctx:
        wpool = ctx.enter_context(tc.tile_pool(name="w", bufs=1))
        xin = ctx.enter_context(tc.tile_pool(name="xin", bufs=24))
        qk = ctx.enter_context(tc.tile_pool(name="qk", bufs=1))
        vpool = ctx.enter_context(tc.tile_pool(name="v", bufs=1))
        ptp = ctx.enter_context(tc.tile_pool(name="pt", bufs=6))
        atp = ctx.enter_context(tc.tile_pool(name="at", bufs=1))
        ypool = ctx.enter_context(tc.tile_pool(name="y", bufs=4))
        misc = ctx.enter_context(tc.tile_pool(name="misc", bufs=3))
        ps = ctx.enter_context(tc.tile_pool(name="ps", bufs=2, space="PSUM"))
        psu = ctx.enter_context(tc.tile_pool(name="psu", bufs=2, space="PSUM"))
        psy = ctx.enter_context(tc.tile_pool(name="psy", bufs=2, space="PSUM"))

        w_sb = {}
        for name in ("wq", "wk", "wv"):
            w_sb[name] = wpool.tile([128, N_DMC * IC], BF16, tag=name,
                                    name=name)
        wo_sb = [wpool.tile([128, DM], BF16, tag=f"wo{i}", name=f"wo{i}")
                 for i in range(2)]

        ones_f32 = misc.tile([128, DH], F32, tag="ones_f32")
        nc.vector.memset(ones_f32[:], 1.0)
        # preload the Exp table while DMAs run
        warm = misc.tile([1, 2], F32, tag="warm")
        nc.vector.memset(warm[:], 0.0)
        nc.scalar.activation(warm[:], warm[:], Exp)

        # ---- long-lived activations ----
        QT = [qk.tile([128, SL], F32R, tag=f"qt{p}", name=f"qt{p}")
              for p in range(2)]
        KT = [qk.tile([128, SL], F32R, tag=f"kt{p}", name=f"kt{p}")
              for p in range(2)]
        AT = [atp.tile([128, SL], BF16, tag=f"at{p}", name=f"at{p}")
              for p in range(2)]
        V = vpool.tile([128, N_KT * VBLK], BF16, tag="vsb")
        for h in range(HPC):
            nc.vector.tensor_copy(V[:, h * VW + 64::VBLK],
                                  ones_f32[:, 0:N_KT])

        for _rep in range(reps):
            # ---- x inputs: contiguous [128,2048] chunk DMAs (4KB rows keep
            # the HW descriptor count low; strided 1KB-run DMAs measured far
            # slower on HW than the sim models). Landing order: kT, qT, vT.
            def x_chunks(xdram, pref):
                ts = []
                for c in range(N_DMC):
                    t = xin.tile([128, SL], BF16, tag="xin",
                                 name=f"{pref}{c}")
                    nc.sync.dma_start(out=t[:],
                                      in_=xdram[c * 128:(c + 1) * 128, :])
                    ts.append(t)
                return ts

            w_dram = {"wq": wqT, "wk": wkT, "wv": wvT}

            def w_load(name):
                if name == "wo":
                    for i in range(2):
                        nc.scalar.dma_start(out=wo_sb[i][:],
                                            in_=woT[i * 128:(i + 1) * 128, :])
                    return
                nc.scalar.dma_start(out=w_sb[name][:], in_=w_dram[name][:])

            w_load("wk")
            w_load("wq")
            w_load("wv")
            w_load("wo")
            kx = x_chunks(kT, "kx")
            qx = x_chunks(qT, "qx")
            vx = x_chunks(vT, "vx")

            # ---- projection group emitters ----
            def qk_group(xt, wname, out_tiles, hp, b):
                acc = psy.tile([128, 512], F32, tag="yp", name=f"acc_{wname}{hp}_{b}")
                for c in range(N_DMC):
                    nc.tensor.matmul(
                        acc[:],
                        w_sb[wname][:, c * IC + hp * 128:
                                    c * IC + (hp + 1) * 128],
                        xt[c][:, b * 512:(b + 1) * 512],
                        start=(c == 0), stop=(c == N_DMC - 1))
                # always DVE: Pool carries partition_broadcasts whose waits
                # would delay a queued proj copy past the next unit's scores
                nc.vector.tensor_copy(out_tiles[hp][:, b * 512:(b + 1) * 512],
                                      acc[:])

            def v_group(kt):
                acc = psy.tile([128, 512], F32, tag="yp", name=f"acc_v{kt}")
                for c in range(N_DMC):
                    nc.tensor.matmul(
                        acc[:, 0:IC],
                        vx[c][:, kt * 128:(kt + 1) * 128],
                        w_sb["wv"][:, c * IC:(c + 1) * IC],
                        start=(c == 0), stop=(c == N_DMC - 1))
                # one strided copy: 4 heads' 64-col blocks into 65-col slots
                nc.vector.tensor_copy(
                    V[:, kt * VBLK:(kt + 1) * VBLK]
                    .rearrange("p (h w) -> p h w", h=HPC)[:, :, 0:64],
                    acc[:, 0:IC].rearrange("p (h w) -> p h w", h=HPC))

            # ---- lead-in projections (rest of K hp0 woven into unit 0) ----
            qk_group(kx, "wk", KT, 0, 0)
            qk_group(qx, "wq", QT, 0, 0)

            def attn_unit(qc, pair, g_insert=None):
                au = [psu.tile([VW, 512], F32, tag="accu", name=f"au{hl}")
                      for hl in range(2)]

                def av_emit(g, pts):
                    for hl in range(2):
                        h = pair * 2 + hl
                        for j in range(2):
                            kt = 2 * g + j
                            nc.tensor.matmul(
                                au[hl][:],
                                V[:, kt * VBLK + h * VW:
                                  kt * VBLK + (h + 1) * VW],
                                pts[hl][:, j * 512:(j + 1) * 512],
                                start=(kt == 0), stop=(kt == N_KT - 1))

                # AV lags scores/exp by one g so the ~1.1us exp latency is
                # hidden behind the next g's scores + inserts
                prev = None
                for g in range(N_KT // 2):
                    pts = []
                    for hl in range(2):
                        s = ps.tile([128, 1024], F32, tag="sgrp")
                        for j in range(2):
                            kt = 2 * g + j
                            nc.tensor.matmul(
                                s[:, j * 512:(j + 1) * 512],
                                KT[pair][hl * 64:(hl + 1) * 64,
                                         kt * 128:(kt + 1) * 128],
                                QT[pair][hl * 64:(hl + 1) * 64,
                                         qc * 512:(qc + 1) * 512],
                                start=True, stop=True)
                        p = ptp.tile([128, 1024], BF16, tag="pt")
                        nc.scalar.activation(p[:], s[:], Exp)
                        pts.append(p)
                    if g_insert is not None:
                        g_insert(g)
                    if prev is not None:
                        av_emit(prev[0], prev[1])
                    prev = (g, pts)
                av_emit(prev[0], prev[1])
                # normalize from PSUM (Pool can't read PSUM on HW, so all
                # PSUM reads stay on DVE). Phased emission (l-copies, recips,
                # pbs, muls) so each engine queue blocks only at its tail.
                # NB: reciprocal_approx_fast reading PSUM directly returns
                # garbage on HW — the l row must be staged through SBUF.
                lsbs, rcs, rbs = [], [], []
                for hl in range(2):
                    l_sb = misc.tile([1, 512], F32, tag="lsb")
                    nc.vector.tensor_copy(l_sb[:], au[hl][64:65, :])
                    lsbs.append(l_sb)
                for hl in range(2):
                    rc = misc.tile([1, 512], F32, tag="rc")
                    nc.vector.reciprocal_approx_fast(out=rc[:],
                                                     in_=lsbs[hl][:])
                    rcs.append(rc)
                for hl in range(2):
                    rb = misc.tile([64, 512], F32, tag="rb")
                    nc.gpsimd.partition_broadcast(rb[:], rcs[hl][:])
                    rbs.append(rb)
                for hl in range(2):
                    nc.vector.tensor_mul(
                        AT[pair][hl * 64:(hl + 1) * 64,
                                 qc * 512:(qc + 1) * 512],
                        au[hl][0:64, :], rbs[hl][:])

            def oproj_qt(qt):
                y_sb = ypool.tile([128, DM], BF16, tag="ysb")
                for mh in range(2):
                    yp = psy.tile([128, 512], F32, tag="yp")
                    for ich in range(2):
                        nc.tensor.matmul(
                            yp[:],
                            AT[ich][:, qt * 128:(qt + 1) * 128],
                            wo_sb[ich][:, mh * 512:(mh + 1) * 512],
                            start=(ich == 0), stop=(ich == 1))
                    nc.vector.tensor_copy(y_sb[:, mh * 512:(mh + 1) * 512],
                                          yp[:])
                nc.sync.dma_start(out=Y[qt * 128:(qt + 1) * 128, :],
                                  in_=y_sb[:])

            # g-indexed PE inserts per unit:
            # unit 0 weaves V proj (paced by vT block DMAs); units 2,3 carry
            # K hp1 groups; units 5..7 carry oproj of qc-1 (one unit of slack
            # after the AT writes they read).
            def u0_ins(g):
                # K hp0 block g+1 must land before scores g=2(g+1) reads it
                if g < 3:
                    qk_group(kx, "wk", KT, 0, g + 1)
                v_group(2 * g)
                v_group(2 * g + 1)

            # Q/K proj groups are emitted at g7 of the PRECEDING unit so
            # their DVE psum-evac copies queue ahead of that unit's
            # normalize chain (otherwise the psy rotation stalls the next
            # unit's oproj/scores behind the chain).
            g_ins = {
                0: {g: [lambda g=g: u0_ins(g)] for g in range(8)},
                2: {4: [lambda: qk_group(kx, "wk", KT, 1, 0)],
                    6: [lambda: qk_group(kx, "wk", KT, 1, 1)]},
                3: {2: [lambda: qk_group(kx, "wk", KT, 1, 2)],
                    5: [lambda: qk_group(kx, "wk", KT, 1, 3)]},
                5: {g: [lambda g=g: oproj_qt(0 * 4 + g // 2)]
                    for g in (0, 2, 4, 6)},
                6: {g: [lambda g=g: oproj_qt(1 * 4 + g // 2)]
                    for g in (0, 2, 4, 6)},
                7: {g: [lambda g=g: oproj_qt(2 * 4 + g // 2)]
                    for g in (0, 2, 4, 6)},
            }
            q_weave = {
                0: lambda: qk_group(qx, "wq", QT, 0, 1),
                1: lambda: qk_group(qx, "wq", QT, 0, 2),
                2: lambda: qk_group(qx, "wq", QT, 0, 3),
                3: lambda: qk_group(qx, "wq", QT, 1, 0),
                4: lambda: qk_group(qx, "wq", QT, 1, 1),
                5: lambda: qk_group(qx, "wq", QT, 1, 2),
                6: lambda: qk_group(qx, "wq", QT, 1, 3),
            }
            for i, fn in q_weave.items():
                g_ins.setdefault(i, {}).setdefault(7, []).append(fn)

            for i in range(8):
                pair, qc = i // 4, i % 4
                gmap = g_ins.get(i, {})
                attn_unit(qc, pair,
                          g_insert=(lambda g, gm=gmap:
                                    [fn() for fn in gm.get(g, ())]))
            for qt in range(12, 16):
                oproj_qt(qt)


_NC_CACHE = None


def _get_nc():
    global _NC_CACHE
    if _NC_CACHE is None:
        _NC_CACHE = build_kernel()
    return _NC_CACHE


def make_in_maps(query, keys, values, Wq, Wk, Wv, Wo):
    query = np.ascontiguousarray(query, dtype=np.float32)
    keys = np.ascontiguousarray(keys, dtype=np.float32)
    values = np.ascontiguousarray(values, dtype=np.float32)
    import ml_dtypes
    bf16 = ml_dtypes.bfloat16
    xTs = {}
    for b in range(BS):
        xTs[b] = (
            np.ascontiguousarray(query[:, b, :].T.astype(bf16)),
            np.ascontiguousarray(keys[:, b, :].T.astype(bf16)),
            np.ascontiguousarray(values[:, b, :].T.astype(bf16)),
        )
    def w_sbuf_layout(wT):
        # [1024 dm, 256 ic] -> SBUF image [128, 8*256]: chunk c at cols c*256
        return np.ascontiguousarray(
            wT.reshape(N_DMC, 128, IC).transpose(1, 0, 2).reshape(128, -1))

    wTs = {}
    for g in range(N_CORES // BS):
        sl = slice(g * IC, (g + 1) * IC)
        wTs[g] = (
            w_sbuf_layout(np.asarray(Wq, np.float32)[sl, :].T.astype(bf16)),
            w_sbuf_layout(np.asarray(Wk, np.float32)[sl, :].T.astype(bf16)),
            w_sbuf_layout(np.asarray(Wv, np.float32)[sl, :].T.astype(bf16)),
            np.ascontiguousarray(np.asarray(Wo, np.float32)[:, sl].T.astype(bf16)),
        )
    in_maps = []
    for c in range(N_CORES):
        b, g = c // 4, c % 4
        qTb, kTb, vTb = xTs[b]
        wq, wk, wv, wo = wTs[g]
        in_maps.append({"qT": qTb, "kT": kTb, "vT": vTb,
                        "wqT": wq, "wkT": wk, "wvT": wv, "woT": wo})
    return in_maps


def assemble_output(results):
    out = np.zeros((SL, BS, DM), dtype=np.float32)
    for c in range(N_CORES):
        b = c // 4
        out[:, b, :] += np.asarray(results[c]["Y"], dtype=np.float32)
    return out


def kernel(query, keys, values, Wq, Wk, Wv, Wo):
    nc = _get_nc()
    in_maps = make_in_maps(query, keys, values, Wq, Wk, Wv, Wo)
    res = run_bass_kernel_spmd(nc, in_maps, list(range(N_CORES)))
    return assemble_output(res.results)


# revision 44
# speedup vs baseline: 1.6739x; 1.3889x over previous
"""Multi-head attention (16 heads, d=64, d_model=1024, SL=2048, BS=2) on 8
Trainium2 NeuronCores.

Sharding: core c handles batch b = c // 4 and heads [4*(c%4), 4*(c%4)+4).
Each core computes a partial output y_c[2048, 1024] (its 4 heads' contribution
through Wo for its batch); the host sums the 4 partials per batch.

Host-side prep feeds activations TRANSPOSED ([d_model, seq]) so every on-chip
matmul has its contraction dim on partitions; no on-chip transposes needed.

Per-core dataflow (all matmul moving streams >= 256 cols so weight loads hide):
  V[2048,256] bf16 with interleaved ones cols ([V_h | 1] per head, 65 cols)
  Q^T,K^T [256,2048] f32r (psum-accumulated over 8 d_model chunks)
  per (q-chunk 512, head): S^T[k,q] psum tiles -> exp (ACT, no max-subtraction:
  |scores| < ~25 so fp32 exp is exact-enough) -> P^T bf16 -> attnU^T[65,512]
  accumulated over 16 k-tiles; row 64 = softmax denominator l.
  evac au->SBUF (frees psum fast), recip(l) -> Pool broadcast -> DVE mul -> A^T
  O-proj: y[q,1024] += A^T-chunk.T @ Wo^T-chunk (wo bf16).

Scheduling (v9):
  - x inputs as contiguous [128,2048] chunk DMAs on the HWDGE (sync) queue
    (4KB rows: HW DMA cost is per-descriptor=per-row, so contiguous large
    rows are far cheaper than strided small-run DMAs); weights pre-laid-out
    on the host to SBUF shape so each loads as ONE contiguous DMA (scalar
    queue). Y out on HWDGE, bf16 (host accumulates partials in f32).
  - emission: K hp0 b0 + Q hp0 b0 lead-in, then 8 attention units
    pair-major. Unit 0's g-loop weaves in the V projection (all 16 k-tiles)
    and the rest of K hp0; K hp1 groups ride units 2-3; each Q proj group is
    emitted at g7 of the unit before its first use (ahead of that unit's
    normalize chain so its PSUM-evac copy isn't queued behind it); oproj(qc)
    tiles are woven into unit (5+qc)'s g-loop, one unit after the AT writes
    they read; oproj(qc3) trails.
  - engines: ALL PSUM evacuations on DVE (Pool cannot touch PSUM on HW);
    Pool only runs partition_broadcasts; normalize emitted in phases
    (l-copies, recips, pbs, muls) so each engine queue blocks only at its
    tail, never mid-chain.
  - PSUM banks: s double-buffered (2x2), au pool 2, proj-acc/oproj pool 2.
  - Exp activation table preloaded at t=0 via a dummy exp.
"""

import os
import sys
for _p in ("/opt/trn_rl_repo", "/root/.axon_site/_ro/trn_rl_repo"):
    if os.path.isdir(_p) and _p not in sys.path:
        sys.path.insert(0, _p)

import numpy as np

import concourse.bass as bass
import concourse.tile as tile
from concourse import bacc, mybir
from concourse.bass_utils import run_bass_kernel_spmd

N_CORES = 8
SL = 2048
BS = 2
DM = 1024          # d_model
H = 16             # total heads
DH = 64            # head dim
HPC = 4            # heads per core
IC = HPC * DH      # per-core inner dim = 256
F32 = mybir.dt.float32
BF16 = mybir.dt.bfloat16
F32R = mybir.dt.float32r
Exp = mybir.ActivationFunctionType.Exp

N_DMC = DM // 128          # 8 d_model chunks
N_KT = SL // 128           # 16 k tiles
N_QC = SL // 512           # 4 q chunks (= x DMA blocks)
VW = 65                    # V columns per head incl. ones column
VBLK = HPC * VW            # 260 V columns per k-tile block


def build_kernel(reps=1):
    nc = bacc.Bacc("TRN2", target_bir_lowering=False, debug=False,
                   num_devices=N_CORES)
    qT = nc.dram_tensor("qT", [DM, SL], BF16, kind="ExternalInput").ap()
    kT = nc.dram_tensor("kT", [DM, SL], BF16, kind="ExternalInput").ap()
    vT = nc.dram_tensor("vT", [DM, SL], BF16, kind="ExternalInput").ap()
    # wq/wk/wv are pre-laid-out on the host in SBUF shape [128, 8*256]
    # (partition-major) so each loads with ONE contiguous 4KB-row DMA
    wqT = nc.dram_tensor("wqT", [128, N_DMC * IC], BF16,
                         kind="ExternalInput").ap()
    wkT = nc.dram_tensor("wkT", [128, N_DMC * IC], BF16,
                         kind="ExternalInput").ap()
    wvT = nc.dram_tensor("wvT", [128, N_DMC * IC], BF16,
                         kind="ExternalInput").ap()
    woT = nc.dram_tensor("woT", [IC, DM], BF16, kind="ExternalInput").ap()
    Y = nc.dram_tensor("Y", [SL, DM], BF16, kind="ExternalOutput").ap()

    with tile.TileContext(nc) as tc:
        _build_body(nc, tc, qT, kT, vT, wqT, wkT, wvT, woT, Y, reps)
    nc.compile()
    return nc


def _build_body(nc, tc, qT, kT, vT, wqT, wkT, wvT, woT, Y, reps=1):
    import contextlib
    ctx = contextlib.ExitStack()
    with ctx:
        wpool = ctx.enter_context(tc.tile_pool(name="w", bufs=1))
        xin = ctx.enter_context(tc.tile_pool(name="xin", bufs=24))
        qk = ctx.enter_context(tc.tile_pool(name="qk", bufs=1))
        vpool = ctx.enter_context(tc.tile_pool(name="v", bufs=1))
        ptp = ctx.enter_context(tc.tile_pool(name="pt", bufs=6))
        atp = ctx.enter_context(tc.tile_pool(name="at", bufs=1))
        ypool = ctx.enter_context(tc.tile_pool(name="y", bufs=4))
        misc = ctx.enter_context(tc.tile_pool(name="misc", bufs=3))
        ps = ctx.enter_context(tc.tile_pool(name="ps", bufs=2, space="PSUM"))
        psu = ctx.enter_context(tc.tile_pool(name="psu", bufs=2, space="PSUM"))
        psy = ctx.enter_context(tc.tile_pool(name="psy", bufs=2, space="PSUM"))

        w_sb = {}
        for name in ("wq", "wk", "wv"):
            w_sb[name] = wpool.tile([128, N_DMC * IC], BF16, tag=name,
                                    name=name)
        wo_sb = [wpool.tile([128, DM], BF16, tag=f"wo{i}", name=f"wo{i}")
                 for i in range(2)]

        ones_f32 = misc.tile([128, DH], F32, tag="ones_f32")
        nc.vector.memset(ones_f32[:], 1.0)
        # preload the Exp table while DMAs run
        warm = misc.tile([1, 2], F32, tag="warm")
        nc.vector.memset(warm[:], 0.0)
        nc.scalar.activation(warm[:], warm[:], Exp)

        # ---- long-lived activations ----
        QT = [qk.tile([128, SL], F32R, tag=f"qt{p}", name=f"qt{p}")
              for p in range(2)]
        KT = [qk.tile([128, SL], F32R, tag=f"kt{p}", name=f"kt{p}")
              for p in range(2)]
        AT = [atp.tile([128, SL], BF16, tag=f"at{p}", name=f"at{p}")
              for p in range(2)]
        V = vpool.tile([128, N_KT * VBLK], BF16, tag="vsb")
        for h in range(HPC):
            nc.vector.tensor_copy(V[:, h * VW + 64::VBLK],
                                  ones_f32[:, 0:N_KT])

        for _rep in range(reps):
            # ---- x inputs: contiguous [128,2048] chunk DMAs (4KB rows keep
            # the HW descriptor count low; strided 1KB-run DMAs measured far
            # slower on HW than the sim models). Landing order: kT, qT, vT.
            def x_chunks(xdram, pref):
                ts = []
                for c in range(N_DMC):
                    t = xin.tile([128, SL], BF16, tag="xin",
                                 name=f"{pref}{c}")
                    nc.sync.dma_start(out=t[:],
                                      in_=xdram[c * 128:(c + 1) * 128, :])
                    ts.append(t)
                return ts

            w_dram = {"wq": wqT, "wk": wkT, "wv": wvT}

            def w_load(name):
                if name == "wo":
                    for i in range(2):
                        nc.scalar.dma_start(out=wo_sb[i][:],
                                            in_=woT[i * 128:(i + 1) * 128, :])
                    return
                nc.scalar.dma_start(out=w_sb[name][:], in_=w_dram[name][:])

            w_load("wk")
            w_load("wq")
            w_load("wv")
            w_load("wo")
            kx = x_chunks(kT, "kx")
            qx = x_chunks(qT, "qx")
            vx = x_chunks(vT, "vx")

            # ---- projection group emitters ----
            def qk_group(xt, wname, out_tiles, hp, b):
                acc = psy.tile([128, 512], F32, tag="yp", name=f"acc_{wname}{hp}_{b}")
                for c in range(N_DMC):
                    nc.tensor.matmul(
                        acc[:],
                        w_sb[wname][:, c * IC + hp * 128:
                                    c * IC + (hp + 1) * 128],
                        xt[c][:, b * 512:(b + 1) * 512],
                        start=(c == 0), stop=(c == N_DMC - 1))
                # always DVE: Pool carries partition_broadcasts whose waits
                # would delay a queued proj copy past the next unit's scores
                nc.vector.tensor_copy(out_tiles[hp][:, b * 512:(b + 1) * 512],
                                      acc[:])

            def v_group(kt):
                acc = psy.tile([128, 512], F32, tag="yp", name=f"acc_v{kt}")
                for c in range(N_DMC):
                    nc.tensor.matmul(
                        acc[:, 0:IC],
                        vx[c][:, kt * 128:(kt + 1) * 128],
                        w_sb["wv"][:, c * IC:(c + 1) * IC],
                        start=(c == 0), stop=(c == N_DMC - 1))
                # one strided copy: 4 heads' 64-col blocks into 65-col slots
                nc.vector.tensor_copy(
                    V[:, kt * VBLK:(kt + 1) * VBLK]
                    .rearrange("p (h w) -> p h w", h=HPC)[:, :, 0:64],
                    acc[:, 0:IC].rearrange("p (h w) -> p h w", h=HPC))

            # ---- lead-in projections (rest of K hp0 woven into unit 0) ----
            qk_group(kx, "wk", KT, 0, 0)
            qk_group(qx, "wq", QT, 0, 0)

            def attn_unit(qc, pair, g_insert=None):
                au = [psu.tile([VW, 512], F32, tag="accu", name=f"au{hl}")
                      for hl in range(2)]

                def av_emit(g, pts):
                    for hl in range(2):
                        h = pair * 2 + hl
                        for j in range(2):
                            kt = 2 * g + j
                            nc.tensor.matmul(
                                au[hl][:],
                                V[:, kt * VBLK + h * VW:
                                  kt * VBLK + (h + 1) * VW],
                                pts[hl][:, j * 512:(j + 1) * 512],
                                start=(kt == 0), stop=(kt == N_KT - 1))

                # AV lags scores/exp by one g so the ~1.1us exp latency is
                # hidden behind the next g's scores + inserts
                prev = None
                for g in range(N_KT // 2):
                    pts = []
                    for hl in range(2):
                        s = ps.tile([128, 1024], F32, tag="sgrp")
                        for j in range(2):
                            kt = 2 * g + j
                            nc.tensor.matmul(
                                s[:, j * 512:(j + 1) * 512],
                                KT[pair][hl * 64:(hl + 1) * 64,
                                         kt * 128:(kt + 1) * 128],
                                QT[pair][hl * 64:(hl + 1) * 64,
                                         qc * 512:(qc + 1) * 512],
                                start=True, stop=True)
                        p = ptp.tile([128, 1024], BF16, tag="pt")
                        nc.scalar.activation(p[:], s[:], Exp)
                        pts.append(p)
                    if g_insert is not None:
                        g_insert(g)
                    if prev is not None:
                        av_emit(prev[0], prev[1])
                    prev = (g, pts)
                av_emit(prev[0], prev[1])
                # normalize from PSUM (Pool can't read PSUM on HW, so all
                # PSUM reads stay on DVE). Phased emission (l-copies, recips,
                # pbs, muls) so each engine queue blocks only at its tail.
                # NB: reciprocal_approx_fast reading PSUM directly returns
                # garbage on HW — the l row must be staged through SBUF.
                lsbs, rcs, rbs = [], [], []
                for hl in range(2):
                    l_sb = misc.tile([1, 512], F32, tag="lsb")
                    nc.vector.tensor_copy(l_sb[:], au[hl][64:65, :])
                    lsbs.append(l_sb)
                for hl in range(2):
                    rc = misc.tile([1, 512], F32, tag="rc")
                    nc.vector.reciprocal_approx_fast(out=rc[:],
                                                     in_=lsbs[hl][:])
                    rcs.append(rc)
                for hl in range(2):
                    rb = misc.tile([64, 512], F32, tag="rb")
                    nc.gpsimd.partition_broadcast(rb[:], rcs[hl][:])
                    rbs.append(rb)
                for hl in range(2):
                    nc.vector.tensor_mul(
                        AT[pair][hl * 64:(hl + 1) * 64,
                                 qc * 512:(qc + 1) * 512],
                        au[hl][0:64, :], rbs[hl][:])

            Copy = mybir.ActivationFunctionType.Copy

            def oproj_qt(qt, tail=False):
                # tail=True (last q-chunk): psum evacuations go on the ACT
                # engine (idle after the final exp) so they overlap the DVE
                # normalize chain
                y_sb = ypool.tile([128, DM], BF16, tag="ysb")
                for mh in range(2):
                    yp = psy.tile([128, 512], F32, tag="yp")
                    for ich in range(2):
                        nc.tensor.matmul(
                            yp[:],
                            AT[ich][:, qt * 128:(qt + 1) * 128],
                            wo_sb[ich][:, mh * 512:(mh + 1) * 512],
                            start=(ich == 0), stop=(ich == 1))
                    if tail:
                        nc.scalar.activation(
                            y_sb[:, mh * 512:(mh + 1) * 512], yp[:], Copy)
                    else:
                        nc.vector.tensor_copy(
                            y_sb[:, mh * 512:(mh + 1) * 512], yp[:])
                nc.sync.dma_start(out=Y[qt * 128:(qt + 1) * 128, :],
                                  in_=y_sb[:])

            # g-indexed PE inserts per unit:
            # unit 0 weaves V proj (paced by vT block DMAs); units 2,3 carry
            # K hp1 groups; units 5..7 carry oproj of qc-1 (one unit of slack
            # after the AT writes they read).
            def u0_ins(g):
                # K hp0 block g+1 must land before scores g=2(g+1) reads it
                if g < 3:
                    qk_group(kx, "wk", KT, 0, g + 1)
                v_group(2 * g)
                v_group(2 * g + 1)

            # Q/K proj groups are emitted at g7 of the PRECEDING unit so
            # their DVE psum-evac copies queue ahead of that unit's
            # normalize chain (otherwise the psy rotation stalls the next
            # unit's oproj/scores behind the chain).
            g_ins = {
                0: {g: [lambda g=g: u0_ins(g)] for g in range(8)},
                2: {4: [lambda: qk_group(kx, "wk", KT, 1, 0)],
                    6: [lambda: qk_group(kx, "wk", KT, 1, 1)]},
                3: {2: [lambda: qk_group(kx, "wk", KT, 1, 2)],
                    5: [lambda: qk_group(kx, "wk", KT, 1, 3)]},
                5: {g: [lambda g=g: oproj_qt(0 * 4 + g // 2)]
                    for g in (0, 2, 4, 6)},
                6: {g: [lambda g=g: oproj_qt(1 * 4 + g // 2)]
                    for g in (0, 2, 4, 6)},
                7: {g: [lambda g=g: oproj_qt(2 * 4 + g // 2)]
                    for g in (0, 2, 4, 6)},
            }
            q_weave = {
                0: lambda: qk_group(qx, "wq", QT, 0, 1),
                1: lambda: qk_group(qx, "wq", QT, 0, 2),
                2: lambda: qk_group(qx, "wq", QT, 0, 3),
                3: lambda: qk_group(qx, "wq", QT, 1, 0),
                4: lambda: qk_group(qx, "wq", QT, 1, 1),
                5: lambda: qk_group(qx, "wq", QT, 1, 2),
                6: lambda: qk_group(qx, "wq", QT, 1, 3),
            }
            for i, fn in q_weave.items():
                g_ins.setdefault(i, {}).setdefault(7, []).append(fn)

            for i in range(8):
                pair, qc = i // 4, i % 4
                gmap = g_ins.get(i, {})
                attn_unit(qc, pair,
                          g_insert=(lambda g, gm=gmap:
                                    [fn() for fn in gm.get(g, ())]))
            for qt in range(12, 16):
                oproj_qt(qt, tail=True)


_NC_CACHE = None


def _get_nc():
    global _NC_CACHE
    if _NC_CACHE is None:
        _NC_CACHE = build_kernel()
    return _NC_CACHE


def make_in_maps(query, keys, values, Wq, Wk, Wv, Wo):
    query = np.ascontiguousarray(query, dtype=np.float32)
    keys = np.ascontiguousarray(keys, dtype=np.float32)
    values = np.ascontiguousarray(values, dtype=np.float32)
    import ml_dtypes
    bf16 = ml_dtypes.bfloat16
    xTs = {}
    for b in range(BS):
        xTs[b] = (
            np.ascontiguousarray(query[:, b, :].T.astype(bf16)),
            np.ascontiguousarray(keys[:, b, :].T.astype(bf16)),
            np.ascontiguousarray(values[:, b, :].T.astype(bf16)),
        )
    def w_sbuf_layout(wT):
        # [1024 dm, 256 ic] -> SBUF image [128, 8*256]: chunk c at cols c*256
        return np.ascontiguousarray(
            wT.reshape(N_DMC, 128, IC).transpose(1, 0, 2).reshape(128, -1))

    wTs = {}
    for g in range(N_CORES // BS):
        sl = slice(g * IC, (g + 1) * IC)
        wTs[g] = (
            w_sbuf_layout(np.asarray(Wq, np.float32)[sl, :].T.astype(bf16)),
            w_sbuf_layout(np.asarray(Wk, np.float32)[sl, :].T.astype(bf16)),
            w_sbuf_layout(np.asarray(Wv, np.float32)[sl, :].T.astype(bf16)),
            np.ascontiguousarray(np.asarray(Wo, np.float32)[:, sl].T.astype(bf16)),
        )
    in_maps = []
    for c in range(N_CORES):
        b, g = c // 4, c % 4
        qTb, kTb, vTb = xTs[b]
        wq, wk, wv, wo = wTs[g]
        in_maps.append({"qT": qTb, "kT": kTb, "vT": vTb,
                        "wqT": wq, "wkT": wk, "wvT": wv, "woT": wo})
    return in_maps


def assemble_output(results):
    out = np.zeros((SL, BS, DM), dtype=np.float32)
    for c in range(N_CORES):
        b = c // 4
        out[:, b, :] += np.asarray(results[c]["Y"], dtype=np.float32)
    return out


def kernel(query, keys, values, Wq, Wk, Wv, Wo):
    nc = _get_nc()
    in_maps = make_in_maps(query, keys, values, Wq, Wk, Wv, Wo)
    res = run_bass_kernel_spmd(nc, in_maps, list(range(N_CORES)))
    return assemble_output(res.results)
